# revision 14
# baseline (speedup 1.0000x reference)
"""Barrier_Net TRN2 kernel v2: 8-core data-parallel Bass/Tile implementation.

Strategy (vs v1 baseline):
  - x shipped feature-major as fp8e4 [85, A] (the MLP path tolerates fp8:
    absmax err ~2e-3 vs tolerance ~4e-2); barrier operands stay f32.
  - Hidden-layer matmuls (16 neighbors + 8 obstacles, H=64) run as fp8
    DoubleRow matmuls (0.5 cyc/row): two k-tiles carry two different
    elements' weights into disjoint output partition halves; rhs supplies
    the same agent columns twice via a duplicated SBUF copy.
  - DeepSet layer-2 sum folds into 6 DoubleRow accumulating matmuls over
    fp8 relu activations (pairs of hidden tiles as k-tiles).
  - rho2 is folded into psi1 (M = rho_w2 @ psi_w1[:2]); the constant
    g0=16 folds into the psi bias; g1's contribution enters via one
    DoubleRow matmul reading x directly.
  - Trunk (rho/psi) runs 2 groups per step stacked on partitions.
  - Barrier + final combine in [128, *] agent-minor layout; empty head
    output transposed via tiny PE transposes; one output DMA.
  - Groups of 512 agents processed in 13 pairs (26 groups, padded 13312).
"""
import sys, os
sys.path.insert(0, "/opt/trn_rl_repo")
import numpy as np
import ml_dtypes
import concourse.bacc as bacc
import concourse.tile as tile
import concourse.mybir as mybir
from concourse.bass_utils import run_bass_kernel_spmd
from contextlib import ExitStack

F32 = mybir.dt.float32
F16 = mybir.dt.float16
F8 = mybir.dt.float8e4
NPF8 = ml_dtypes.float8_e4m3
AF = mybir.ActivationFunctionType
ALU = mybir.AluOpType
DR = mybir.MatmulPerfMode.DoubleRow

B, NN, NO, SD = 100000, 16, 8, 4
H, PHI_OUT, ADIM = 64, 16, 2
DS, B_GAMMA = 0.2, 0.01
D_OBS = 85
NCORE = 8
AC = B // NCORE            # 12500 agents per core
NG = 26                    # groups of 512 (padded)
A2 = NG * 512              # 13312
NP = NG // 2               # 13 pairs


def _pack_weights(phi_w1, phi_b1, phi_w2, phi_b2, obs_w1, obs_b1, obs_w2, obs_b2,
                  rho_w1, rho_b1, rho_w2, rho_b2, psi_w1, psi_b1, psi_w2, psi_b2):
    # hidden-layer DoubleRow weights: [85, 2, 128] per block, flattened
    w1dr = np.zeros((85, 8, 2, 128), np.float32)
    for k in range(8):
        for s in range(4):
            w1dr[5 + 8 * k + s, k, 0, 0:64] = phi_w1[s]
            w1dr[9 + 8 * k + s, k, 1, 64:128] = phi_w1[s]
    ow1dr = np.zeros((85, 4, 2, 128), np.float32)
    for m in range(4):
        for s in range(2):
            ow1dr[69 + 4 * m + s, m, 0, 0:64] = obs_w1[s]
            ow1dr[71 + 4 * m + s, m, 1, 64:128] = obs_w1[s]
    # fused layer2+rho1 DoubleRow weights [128, 2, 64]: CW = tile(w2) @ rho_w1
    CWP = np.zeros((128, 64), np.float32)
    CWP[0:64] = phi_w2 @ rho_w1; CWP[64:128] = phi_w2 @ rho_w1
    CWO = np.zeros((128, 64), np.float32)
    CWO[0:64] = obs_w2 @ rho_w1; CWO[64:128] = obs_w2 @ rho_w1
    w2dr = np.stack([CWP, CWP], axis=1)     # [128,2,64]
    ow2dr = np.stack([CWO, CWO], axis=1)
    # psi g-term DoubleRow lhsT [85, 2, 128]: row 1 (= feature g1)
    gdr = np.zeros((85, 2, 128), np.float32)
    gdr[1, 0, 0:64] = psi_w1[3]
    gdr[1, 1, 64:128] = psi_w1[3]
    # trunk f16 weights, 2-group stacked
    M = rho_w2 @ psi_w1[0:2]                  # [64, 64]
    ms = np.zeros((128, 128), np.float32)
    ms[0:64, 0:64] = M; ms[64:128, 64:128] = M
    pw2s = np.zeros((128, 36), np.float32)
    pw2s[0:64, 0:2] = psi_w2; pw2s[64:128, 32:34] = psi_w2
    # bias columns
    small = np.zeros((128, 8), np.float32)
    small[:, 0] = np.tile(phi_b1, 2)
    small[:, 1] = np.tile(obs_b1, 2)
    rb = rho_b1 + (NN * phi_b2 + NO * obs_b2) @ rho_w1
    small[:, 2] = np.tile(rb, 2)
    pb = psi_b1 + rho_b2 @ psi_w1[0:2] + 16.0 * psi_w1[2]
    small[:, 3] = np.tile(pb, 2)
    small[0:2, 4] = psi_b2
    small[32:34, 4] = psi_b2
    wf8 = np.zeros((128, 3840), np.float32)
    wf8[0:85, 0:2048] = w1dr.reshape(85, 2048)
    wf8[0:85, 2048:3072] = ow1dr.reshape(85, 1024)
    wf8[:, 3072:3200] = w2dr.reshape(128, 128)
    wf8[:, 3200:3328] = ow2dr.reshape(128, 128)
    wf8[0:85, 3328:3584] = gdr.reshape(85, 256)
    wf16 = np.zeros((128, 164), np.float32)
    wf16[:, 0:128] = ms
    wf16[:, 128:164] = pw2s
    wf32 = np.zeros((128, 12), np.float32)
    wf32[:, 0:8] = small
    wf32[0:2, 8:10] = np.eye(2)
    wf32[32:34, 8:10] = np.eye(2)
    return dict(wf8=wf8.astype(NPF8), wf16=wf16.astype(np.float16), wf32=wf32)


def _build(nc):
    xt8_d = nc.dram_tensor("xt8", [85, A2], F8, kind="ExternalInput").ap()
    xbi_d = nc.dram_tensor("xbi", [128, NG * 128], F32, kind="ExternalInput").ap()
    wf8_d = nc.dram_tensor("wf8", [128, 3840], F8, kind="ExternalInput").ap()
    wf16_d = nc.dram_tensor("wf16", [128, 164], F16, kind="ExternalInput").ap()
    wf32_d = nc.dram_tensor("wf32", [128, 12], F32, kind="ExternalInput").ap()
    y_d = nc.dram_tensor("y", [128, NG * 8], F32, kind="ExternalOutput").ap()

    with tile.TileContext(nc) as tc, ExitStack() as ctx:
        cw = ctx.enter_context(tc.tile_pool(name="cw", bufs=1))
        xin = ctx.enter_context(tc.tile_pool(name="xin", bufs=3))
        ev = ctx.enter_context(tc.tile_pool(name="ev", bufs=2))
        sm = ctx.enter_context(tc.tile_pool(name="sm", bufs=3))
        bp = ctx.enter_context(tc.tile_pool(name="bp", bufs=2))
        p1 = ctx.enter_context(tc.tile_pool(name="p1", bufs=3, space="PSUM"))
        p2 = ctx.enter_context(tc.tile_pool(name="p2", bufs=1, space="PSUM"))
        p3 = ctx.enter_context(tc.tile_pool(name="p3", bufs=1, space="PSUM"))

        wf8t = cw.tile([128, 3840], F8); nc.sync.dma_start(wf8t[:], wf8_d)
        wf16t = cw.tile([128, 164], F16); nc.sync.dma_start(wf16t[:], wf16_d)
        wf32t = cw.tile([128, 12], F32); nc.sync.dma_start(wf32t[:], wf32_d)
        w1p = wf8t[0:85, 0:2048]
        ow1p = wf8t[0:85, 2048:3072]
        w2p = wf8t[:, 3072:3200]
        ow2p = wf8t[:, 3200:3328]
        gp8 = wf8t[0:85, 3328:3584]
        mt = wf16t[:, 0:128]
        pw2t = wf16t[:, 128:164]
        smallt = wf32t[:, 0:8]
        id2 = wf32t[:, 8:10]
        ybuf = cw.tile([128, NG * 8], F32)
        eTall = cw.tile([128, NG * 8], F32)
        barall = cw.tile([128, NG * 8], F32)

        for t in range(NP):
            cs = t * 1024
            xt = xin.tile([85, 2048], F8)
            nc.sync.dma_start(xt[:, 0:1024], xt8_d[:, cs:cs + 1024])
            nc.sync.dma_start(xt[:, 1024:2048], xt8_d[:, cs:cs + 1024])
            xb = xin.tile([128, 256], F32)
            nc.gpsimd.dma_start(xb[:], xbi_d[:, t * 256:t * 256 + 256])
            # view: [85, copy(2), group(2), 512]
            xtv = xt[:].rearrange("p (i g n) -> p i g n", i=2, g=2)

            # ---- barrier (issued early; runs on Pool/DVE/Act(Sqrt)) ----
            sq = bp.tile([128, 256], F32, tag="sq")
            nc.gpsimd.tensor_mul(sq[:], xb[:], xb[:])
            sqv = sq[:].rearrange("p (x u) -> p x u", u=2)
            n2 = bp.tile([128, 128], F32, tag="n2")
            nc.gpsimd.tensor_tensor(out=n2[:], in0=sqv[:, :, 0],
                                    in1=sqv[:, :, 1], op=ALU.add)
            dsr = bp.tile([128, 128], F32, tag="dsr")
            nc.scalar.activation(dsr[:], n2[:], AF.Sqrt)
            dd = bp.tile([128, 128], F32, tag="dd")
            nc.gpsimd.tensor_scalar(dd[:], dsr[:], -DS, 1.0 / B_GAMMA,
                                    op0=ALU.add, op1=ALU.mult)
            rr = bp.tile([128, 128], F32, tag="rr")
            nc.vector.reciprocal_approx_fast(out=rr[:], in_=dd[:])
            rp = bp.tile([128, 256], F32, tag="rp")
            rrb = rr[:].unsqueeze(2).broadcast_to([128, 128, 2])
            nc.gpsimd.tensor_tensor(
                out=rp[:].rearrange("p (x u) -> p x u", u=2),
                in0=xb[:].rearrange("p (x u) -> p x u", u=2),
                in1=rrb, op=ALU.mult)
            nc.vector.tensor_reduce(
                out=barall[:, 16 * t:16 * t + 16].rearrange(
                    "p (b u) -> p b u", u=2),
                in_=rp[:].rearrange("p (b n u) -> p b u n", n=16, u=2),
                axis=mybir.AxisListType.X, op=ALU.add)

            sev = ev.tile([128, 12 * 1024], F8)
            sevv = sev[:].rearrange("p (j n) -> p j n", j=12)
            for j in range(12):
                ps = p1.tile([128, 1024], F32)
                if j < 8:
                    lhs = w1p[:, 256 * j:256 * j + 256]
                    bcol = smallt[:, 0:1]
                else:
                    lhs = ow1p[:, 256 * (j - 8):256 * (j - 8) + 256]
                    bcol = smallt[:, 1:2]
                lhsv = lhs.rearrange("p (i m) -> p i m", i=2)
                nc.tensor.matmul(ps[:, 0:512], lhsT=lhsv, rhs=xtv[:, :, 0, :],
                                 start=True, stop=True, perf_mode=DR)
                nc.tensor.matmul(ps[:, 512:1024], lhsT=lhsv, rhs=xtv[:, :, 1, :],
                                 start=True, stop=True, perf_mode=DR)
                if j % 2 == 0:
                    nc.scalar.activation(sevv[:, j, :], ps[:], AF.Relu, bias=bcol)
                else:
                    nc.vector.tensor_scalar(sevv[:, j, :], ps[:], bcol, 0.0,
                                            op0=ALU.add, op1=ALU.max)

            # ---- fused deepset-L2 + rho1: 12 DR matmuls into prhA/prhB ----
            prhA = p3.tile([64, 512], F32, tag="tk")
            prhB = p2.tile([64, 512], F32, tag="phb")
            for jj in range(6):
                j = 2 * jj + 1
                l2 = (w2p if j < 8 else ow2p).rearrange("p (i m) -> p i m", i=2)
                st, sp = (jj == 0), (jj == 5)
                nc.tensor.matmul(prhA[:], lhsT=l2,
                                 rhs=sevv[:, j - 1:j + 1, 0:512],
                                 start=st, stop=sp, perf_mode=DR)
                nc.tensor.matmul(prhB[:], lhsT=l2,
                                 rhs=sevv[:, j - 1:j + 1, 512:1024],
                                 start=st, stop=sp, perf_mode=DR)
            rhS = sm.tile([128, 512], F16, tag="rh")
            nc.scalar.activation(rhS[0:64, :], prhA[:], AF.Relu,
                                 bias=smallt[0:64, 2:3])
            nc.vector.tensor_scalar(rhS[64:128, :], prhB[:], smallt[64:128, 2:3],
                                    0.0, op0=ALU.add, op1=ALU.max)
            ppsi = p3.tile([128, 512], F32, tag="tk")
            nc.tensor.matmul(ppsi[:], lhsT=gp8.rearrange("p (i m) -> p i m", i=2),
                             rhs=xt[:, 0:1024].rearrange("p (i n) -> p i n", i=2),
                             start=True, stop=False, perf_mode=DR)
            nc.tensor.matmul(ppsi[:], lhsT=mt, rhs=rhS[:],
                             start=False, stop=True)
            phhS = sm.tile([128, 512], F16, tag="phh")
            nc.scalar.activation(phhS[:], ppsi[:], AF.Relu, bias=smallt[:, 3:4])
            pe2 = p3.tile([36, 512], F32, tag="tk")
            nc.tensor.matmul(pe2[:], lhsT=pw2t, rhs=phhS[:],
                             start=True, stop=True)
            esbS = sm.tile([36, 512], F32, tag="esb")
            nc.scalar.activation(esbS[:], pe2[:], AF.Identity, bias=smallt[0:36, 4:5])
            eTp = p3.tile([128, 16], F32, tag="tk")
            for c8 in range(8):
                gl, c = c8 // 4, c8 % 4
                nc.tensor.transpose(eTp[:, 2 * c8:2 * c8 + 2],
                                    esbS[32 * gl:32 * gl + 2, 128 * c:128 * c + 128],
                                    id2[32 * gl:32 * gl + 2, :])

            nc.vector.tensor_copy(eTall[:, 16 * t:16 * t + 16], eTp[:])

        # ---- deferred combine: y = 2*tanh(tanh(e) + bar), one Tanh table load
        e1 = cw.tile([128, NG * 8], F32)
        nc.scalar.activation(e1[:], eTall[:], AF.Tanh)
        yt = cw.tile([128, NG * 8], F32)
        nc.vector.tensor_add(yt[:], e1[:], barall[:])
        ya = cw.tile([128, NG * 8], F32)
        nc.scalar.activation(ya[:], yt[:], AF.Tanh)
        nc.gpsimd.tensor_scalar_mul(ybuf[:], ya[:], 2.0)
        nc.sync.dma_start(y_d, ybuf[:])
    return nc


_CACHED = {}


def kernel(**inputs):
    x = np.asarray(inputs["x"], np.float32)
    wk = _pack_weights(**{k: np.asarray(v, np.float32) for k, v in inputs.items()
                          if k != "x"})
    in_maps = []
    for cidx in range(NCORE):
        xs = x[cidx * AC:(cidx + 1) * AC]
        xp = np.zeros((A2, D_OBS), np.float32)
        xp[:AC] = xs
        m = dict(wk)
        m["xt8"] = np.ascontiguousarray(xp.T).astype(NPF8)
        nbv = -xp[:, 5:69].reshape(A2, 16, 4)[:, :, 0:2]      # [A2, 16, 2]
        m["xbi"] = np.ascontiguousarray(
            nbv.reshape(NG, 4, 128, 32).transpose(2, 0, 1, 3).reshape(128, NG * 128))
        in_maps.append(m)

    if "nc" not in _CACHED:
        nc = bacc.Bacc("TRN2", target_bir_lowering=False, debug=False,
                       num_devices=NCORE)
        _build(nc)
        nc.compile()
        _CACHED["nc"] = nc
    nc = _CACHED["nc"]
    res = run_bass_kernel_spmd(nc, in_maps, core_ids=list(range(NCORE)))
    _CACHED["res"] = res
    out = np.empty((B, ADIM), np.float32)
    for cidx in range(NCORE):
        Y = res.results[cidx]["y"]                            # [128, NG*8]
        Y4 = Y.reshape(128, NG, 4, 2).transpose(1, 2, 0, 3).reshape(A2, 2)
        out[cidx * AC:(cidx + 1) * AC] = Y4[:AC]
    return out


if __name__ == "__main__":
    import reference
    ins = {k: np.asarray(v) for k, v in reference.setup_inputs().items()}
    got = kernel(**ins)
    exp = np.asarray(reference.reference(**ins))
    err = np.abs(got - exp).max()
    rel = err / np.abs(exp).max()
    print(f"absmax {err:.4e} rel {rel:.4e}")
